# revision 14
# baseline (speedup 1.0000x reference)
"""ContiFormer-style transformer kernel for 8 Trainium2 cores.

Sharding: data-parallel over batch (B=8 -> 1 batch element per core).
All parameters replicated (tiny, d_model=64).

Device kernel design (per core, T=2048, d=64, H=4, dk=16, L=3):
  - Residual h kept transposed in SBUF: hT [64, 2048]  (d on partitions).
  - RK4 ODE evolution of k/v is linear (f(u)=u@(S-S^T)), so the whole
    evolution collapses to a per-head 16x16 matrix M = P^4 with
    P = I + dtA + (dtA)^2/2 + (dtA)^3/6 + (dtA)^4/24, folded into the
    K/V projection weights on the host. The 1/sqrt(dk) scale is folded
    into Wq. Biases are folded into the matmuls via a persistent ones
    row in the y buffer (and K=1 accumulate matmuls for bo/b2).
  - Attention in S^T form: S^T[m,q] = sum_dk kT[dk,m] * qT[dk,q] per
    head; p = exp(S^T) (no max subtraction: logits are O(6)); the
    softmax denominator Z comes from a ones column appended to the
    token-major V tile, so mm2 produces [o_unnorm; Z] together.
  - LayerNorm stats via ones-matmuls on PE (mean/var broadcast over
    partitions for free); rstd computed as exp(-0.5*ln(var+eps)) so the
    only ACT tables used are ln/exp (+relu/copy) -> no table switches.
  - Only the last token is needed after layer 2, so layer 3 computes
    q/attention/MLP for the last 8 queries only.
"""

import os
import numpy as np

N_LAYERS, D_MODEL, N_HEAD = 3, 64, 4
DK = D_MODEL // N_HEAD            # 16
D_INNER = 2 * D_MODEL             # 128
N_FEAT, SEQ_LEN, BATCH = 16, 2048, 8
RK4_STEPS = 4
N_CORES = 8
T = SEQ_LEN
MB = 128                          # key block (m) size
N_MB = T // MB                    # 16
QC = 512                          # query chunk for full layers
VAW = 128                         # per-mb Vaug stride: 4 heads x 32 cols (15 zero)
EPS = 1e-5

_PROGRAM = None                   # cached (nc, input_names)


def _rk4_matrix(S):
    """S: [H, dk, dk] float64 -> M = P^4 per head, P = deg-4 Taylor of exp(dt*A)."""
    A = S - np.swapaxes(S, -1, -2)
    dt = 1.0 / RK4_STEPS
    I = np.eye(DK)[None]
    dA = dt * A
    P = I + dA + dA @ dA / 2.0 + dA @ dA @ dA / 6.0 + dA @ dA @ dA @ dA / 24.0
    M = P @ P
    M = M @ M
    return M                      # [H, dk, dk]


def _blockdiag(Ms):
    """[H, dk, dk] -> [64, 64] block diagonal."""
    out = np.zeros((D_MODEL, D_MODEL), dtype=Ms.dtype)
    for h in range(N_HEAD):
        out[h * DK:(h + 1) * DK, h * DK:(h + 1) * DK] = Ms[h]
    return out


def _spread_cols(W):
    """[.., 64] -> [.., 128] with head h's 16 cols moved to 32h..32h+16."""
    out = np.zeros(W.shape[:-1] + (128,), dtype=W.dtype)
    for h in range(N_HEAD):
        out[..., 32 * h:32 * h + DK] = W[..., DK * h:DK * (h + 1)]
    return out


def _spread_rows(W):
    """[64, ..] -> [128, ..] with head h's 16 rows moved to 32h..32h+16."""
    out = np.zeros((128,) + W.shape[1:], dtype=W.dtype)
    for h in range(N_HEAD):
        out[32 * h:32 * h + DK] = W[DK * h:DK * (h + 1)]
    return out


def _host_weights(inp):
    """Precompute all effective weight tensors (numpy float32)."""
    f64 = lambda a: np.asarray(a, dtype=np.float64)
    w = {}
    w["W_in_aug"] = np.vstack([f64(inp["W_in"]), f64(inp["b_in"])[None]])  # [17, 64]
    for i in range(N_LAYERS):
        Mk = _blockdiag(_rk4_matrix(f64(inp["Sk"][i])))
        Mv = _blockdiag(_rk4_matrix(f64(inp["Sv"][i])))
        Wq = f64(inp["Wq"][i]) * 0.25
        bq = f64(inp["bq"][i]) * 0.25
        Wk = f64(inp["Wk"][i]) @ Mk
        bk = f64(inp["bk"][i]) @ Mk
        Wv = f64(inp["Wv"][i]) @ Mv
        bv = f64(inp["bv"][i]) @ Mv
        w[f"Wq_aug_{i}"] = _spread_cols(np.vstack([Wq, bq[None]]))   # [65, 128]
        w[f"Wk_aug_{i}"] = _spread_cols(np.vstack([Wk, bk[None]]))   # [65, 128]
        w[f"Wv_aug_{i}"] = _spread_cols(np.vstack([Wv, bv[None]]))   # [65, 128]
        w[f"Wo_sp_{i}"] = _spread_rows(f64(inp["Wo"][i]))            # [128, 64]
        w[f"bo_row_{i}"] = f64(inp["bo"][i])[None]                   # [1, 64]
        w[f"W1_aug_{i}"] = np.vstack([f64(inp["W1"][i]), f64(inp["b1"][i])[None]])  # [65, 128]
        w[f"W2_{i}"] = f64(inp["W2"][i])                             # [128, 64]
        w[f"b2_row_{i}"] = f64(inp["b2"][i])[None]                   # [1, 64]
        w[f"ln1_s_{i}"] = f64(inp["ln1_s"][i])[:, None]              # [64, 1]
        w[f"ln1_b_{i}"] = f64(inp["ln1_b"][i])[:, None]
        w[f"ln2_s_{i}"] = f64(inp["ln2_s"][i])[:, None]
        w[f"ln2_b_{i}"] = f64(inp["ln2_b"][i])[:, None]
    w["lnf_s"] = f64(inp["lnf_s"])[:, None]
    w["lnf_b"] = f64(inp["lnf_b"])[:, None]
    w["Wh1_aug"] = np.vstack([f64(inp["Wh1"]), f64(inp["bh1"])[None]])  # [65, 32]
    w["Wh2_aug"] = np.vstack([f64(inp["Wh2"]), f64(inp["bh2"])[None]])  # [33, 1]

    # constant matrices
    Cmat = np.eye(D_MODEL) - np.full((D_MODEL, D_MODEL), 1.0 / D_MODEL)  # centering
    Dmat = np.full((D_MODEL, D_MODEL), 1.0 / D_MODEL)                    # mean bcast
    w["Cmat"] = Cmat
    w["Dmat"] = Dmat
    selZ = np.zeros((128, N_HEAD))
    for h in range(N_HEAD):
        selZ[32 * h + DK, h] = 1.0                                       # Z row pick
    w["selZ"] = selZ
    B4 = np.zeros((N_HEAD, 128))
    for h in range(N_HEAD):
        B4[h, 32 * h:32 * h + DK] = 1.0                                  # zinv bcast
    w["B4"] = B4
    return {k: np.ascontiguousarray(v, dtype=np.float32) for k, v in w.items()}


def _build_program(weight_shapes):
    import concourse.bacc as bacc
    import concourse.tile as tile
    from concourse import mybir
    from concourse.masks import make_identity
    from concourse._compat import axon_active

    FP = mybir.dt.float32
    FR = mybir.dt.float32r
    BF = mybir.dt.bfloat16
    AF = mybir.ActivationFunctionType
    ALU = mybir.AluOpType
    # ln scale/bias + eps stay fp32 (scalar operands); everything that feeds
    # a matmul is typed float32r so the producing instruction rounds it.
    FP_WEIGHTS = {"ln1_s_", "ln1_b_", "ln2_s_", "ln2_b_", "lnf_s", "lnf_b", "Wh2_aug"}

    def wdt_of(name):
        return FP if any(name.startswith(p) for p in FP_WEIGHTS) else FR

    nc = bacc.Bacc("TRN2", target_bir_lowering=False, debug=not axon_active())

    # All activation funcs used here (Ln, Exp, Relu) live together in the
    # natural_log_exp_and_others table, but the default table chooser picks
    # the first table containing each func (exp_and_others for Exp,
    # natural_log for Ln), forcing a ~1.3us table reload on every switch.
    # Steer it by hiding Ln/Exp in every other table; list order (and thus
    # act_func_set_id) is preserved, and the chosen table genuinely contains
    # all funcs, so the emitted program is valid.
    def _patched_table_loads():
        import bass_rust as _bass_rust
        from concourse.hw_specs import get_activation_tables
        has_activation = any(
            isinstance(ins, mybir.InstActivation)
            for b in nc.main_func.blocks
            for ins in b.instructions
        )
        if not has_activation:
            return
        keep = {mybir.ActivationFunctionType.Ln, mybir.ActivationFunctionType.Exp}
        tables = []
        for name, funcs in get_activation_tables(nc.m.arch).items():
            if name != "natural_log_exp_and_others":
                funcs = funcs - keep
            tables.append((name, funcs))
        _bass_rust.insert_act_table_loads(nc, tables)

    nc.insert_act_table_loads = _patched_table_loads
    dram = {}
    dram["x_aug"] = nc.dram_tensor("x_aug", [N_FEAT + 1, T], FR, kind="ExternalInput").ap()
    for name, shp in weight_shapes.items():
        dram[name] = nc.dram_tensor(name, list(shp), wdt_of(name), kind="ExternalInput").ap()
    out_dram = nc.dram_tensor("out", [1, 1], FP, kind="ExternalOutput").ap()

    with tile.TileContext(nc) as tc:
        with (
            tc.tile_pool(name="state", bufs=1) as state,
            tc.tile_pool(name="wpool", bufs=1) as wpool,
            tc.tile_pool(name="sb", bufs=3) as sb,
            tc.tile_pool(name="ppool", bufs=2) as ppool,
        ):
            # ---- load weights to SBUF ----
            wsb = {}
            for name, shp in weight_shapes.items():
                t = wpool.tile(list(shp), wdt_of(name), tag=name)
                nc.sync.dma_start(t[:], dram[name][:])
                wsb[name] = t
            xa = state.tile([N_FEAT + 1, T], FR, tag="xa")
            nc.sync.dma_start(xa[:], dram["x_aug"][:])

            ident = state.tile([128, 128], FP, tag="ident")
            make_identity(nc, ident[:])

            # ---- persistent state ----
            hT = state.tile([D_MODEL, T], FR, tag="hT")
            y_aug = state.tile([D_MODEL + 1, T], FR, tag="y_aug")
            ones_fp = state.tile([1, T], FP, tag="ones_fp")
            nc.vector.memset(ones_fp[:], 1.0)
            nc.vector.tensor_copy(y_aug[D_MODEL:D_MODEL + 1, :], ones_fp[:])
            q_sp = state.tile([128, T], FR, tag="q_sp")
            k_sp = state.tile([128, T], FR, tag="k_sp")
            v_sp = state.tile([128, T], FR, tag="v_sp")
            vaug = state.tile([128, N_MB * VAW], BF, tag="vaug")
            nc.vector.memset(vaug[:], 0.0)
            r_mlp = state.tile([D_INNER, T], FR, tag="r_mlp")
            q3 = state.tile([128, 8], FR, tag="q3")
            eps64 = state.tile([D_MODEL, 1], FP, tag="eps64")
            nc.vector.memset(eps64[:], EPS)
            ones_row = state.tile([1, T], FR, tag="ones_row")
            nc.vector.tensor_copy(ones_row[:], ones_fp[:])

            def matmul(out, lhsT, rhs, **kw):
                # Matmul operands are float32r-typed tiles: same bits as fp32
                # but streams at 1 PE cycle/row (vs 4 for fp32) once the
                # moving dim is >= 256. Below that the PE cost is the same as
                # fp32, so use plain fp32 and dodge fp32r ISA restrictions.
                if rhs.dtype == FR and rhs.free_size() < 256:
                    lhsT = lhsT.bitcast(FP) if lhsT.dtype == FR else lhsT
                    rhs = rhs.bitcast(FP)
                nc.tensor.matmul(out, lhsT, rhs, **kw)

            # ---- h = W_in_aug^T @ x_aug ----
            with tc.tile_pool(name="h0ps", bufs=2, space="PSUM") as h0ps:
                for c in range(T // QC):
                    ps = h0ps.tile([D_MODEL, QC], FP, tag="proj")
                    matmul(ps[:], wsb["W_in_aug"][:], xa[:, c * QC:(c + 1) * QC],
                           start=True, stop=True)
                    nc.vector.tensor_copy(hT[:, c * QC:(c + 1) * QC], ps[:])

            def layernorm(src, s_ap, b_ap, lo, width):
                """LN over d of src[64, lo:lo+width] -> y_aug[0:64, lo:lo+width]."""
                with tc.tile_pool(name="lnps", bufs=2, space="PSUM") as lnps:
                    nch = (width + QC - 1) // QC
                    for c in range(nch):
                        o, wdt = lo + c * QC, min(QC, width - c * QC)
                        hc_t = lnps.tile([D_MODEL, QC], FP, tag="ln_hc")
                        hc = hc_t[:, 0:wdt]
                        matmul(hc, wsb["Cmat"][:], src[:, o:o + wdt],
                               start=True, stop=True)
                        hcs = sb.tile([D_MODEL, wdt], FP, tag="ln_hcs")
                        nc.vector.tensor_copy(hcs[:], hc)
                        sq = sb.tile([D_MODEL, wdt], FR, tag="ln_sq")
                        nc.vector.tensor_mul(sq[:], hcs[:], hcs[:])
                        var_t = lnps.tile([D_MODEL, QC], FP, tag="ln_var")
                        var = var_t[:, 0:wdt]
                        matmul(var, wsb["Dmat"][:], sq[:], start=True, stop=True)
                        lnv = sb.tile([D_MODEL, wdt], FP, tag="ln_lnv")
                        nc.scalar.activation(lnv[:], var, AF.Ln, bias=eps64[:])
                        rstd = sb.tile([D_MODEL, wdt], FP, tag="ln_rstd")
                        nc.scalar.activation(rstd[:], lnv[:], AF.Exp, scale=-0.5)
                        yn = sb.tile([D_MODEL, wdt], FP, tag="ln_yn")
                        nc.vector.tensor_mul(yn[:], hcs[:], rstd[:])
                        nc.vector.tensor_scalar(y_aug[0:D_MODEL, o:o + wdt], yn[:],
                                                s_ap, b_ap, op0=ALU.mult, op1=ALU.add)

            def project(wname, dst, lo, width, dst_lo=None):
                """dst[:, dst_lo:+width] = (W^T @ y_aug)[:, lo:lo+width]."""
                if dst_lo is None:
                    dst_lo = lo
                M = wsb[wname].shape[-1]
                with tc.tile_pool(name="prps", bufs=2, space="PSUM") as prps:
                    nch = (width + QC - 1) // QC
                    for c in range(nch):
                        o, wdt = c * QC, min(QC, width - c * QC)
                        ps = prps.tile([M, QC], FP, tag="proj", name="proj")[:, 0:wdt]
                        matmul(ps, wsb[wname][:],
                               y_aug[:, lo + o:lo + o + wdt], start=True, stop=True)
                        nc.vector.tensor_copy(dst[:, dst_lo + o:dst_lo + o + wdt], ps)

            def build_vaug():
                with tc.tile_pool(name="tps", bufs=2, space="PSUM") as tps:
                    for mb in range(N_MB):
                        pt = tps.tile([128, 128], FP, tag="vt")
                        nc.tensor.transpose(pt[:],
                                            v_sp[:, mb * MB:(mb + 1) * MB].bitcast(FP),
                                            ident[:])
                        src = pt[:, 0:128].rearrange(
                            "p (h c) -> p h c", h=N_HEAD)[:, :, 0:DK]
                        dst = vaug[:, mb * VAW:(mb + 1) * VAW].rearrange(
                            "p (h c) -> p h c", h=N_HEAD)[:, :, 0:DK]
                        nc.vector.tensor_copy(dst, src)
                        ones_dst = vaug[:, mb * VAW:(mb + 1) * VAW].rearrange(
                            "p (h c) -> p h c", h=N_HEAD)[:, :, DK:DK + 1]
                        nc.vector.memset(ones_dst, 1.0)  # cols DK+1..31 stay zero

            def attention(i, qsrc, qlo, qw, out_lo):
                """h[:, out_lo:+qw] += (softmax(qk^T)v @ Wo + bo) for queries
                [qlo, qlo+qw) of qsrc (spread layout)."""
                with (
                    tc.tile_pool(name="spool", bufs=2, space="PSUM") as spool,
                    tc.tile_pool(name="opool", bufs=2, space="PSUM") as opool,
                    tc.tile_pool(name="epps", bufs=2, space="PSUM") as epps,
                ):
                    n_qc = (qw + QC - 1) // QC
                    for c in range(n_qc):
                        qo, cw = qlo + c * QC, min(QC, qw - c * QC)
                        ops = opool.tile([128, QC], FP, tag="o_ps", name="o_ps")[:, 0:cw]
                        for mb in range(N_MB):
                            for pair in range(2):
                                st = spool.tile([128, 2 * QC], FP, tag="s_ps")
                                for hh in range(2):
                                    h = 2 * pair + hh
                                    matmul(st[:, hh * QC:hh * QC + cw],
                                           k_sp[32 * h:32 * h + DK,
                                                mb * MB:(mb + 1) * MB],
                                           qsrc[32 * h:32 * h + DK, qo:qo + cw],
                                           start=True, stop=True,
                                           tile_position=(32 * h, 0))
                                pt = ppool.tile([128, 2 * cw], BF, tag="p_sb")
                                if cw == QC:
                                    nc.scalar.activation(pt[:], st[:], AF.Exp)
                                else:
                                    for hh in range(2):
                                        nc.scalar.activation(
                                            pt[:, hh * cw:(hh + 1) * cw],
                                            st[:, hh * QC:hh * QC + cw], AF.Exp)
                                for hh in range(2):
                                    h = 2 * pair + hh
                                    matmul(ops[32 * h:32 * h + 32, :],
                                           vaug[:, mb * VAW + h * 32:
                                                mb * VAW + (h + 1) * 32],
                                           pt[:, hh * cw:(hh + 1) * cw],
                                           start=(mb == 0), stop=(mb == N_MB - 1),
                                           tile_position=(0, 32 * h),
                                           skip_group_check=True)
                        # epilogue for this q chunk
                        osb = sb.tile([128, cw], FR, tag="o_sb")
                        nc.vector.tensor_copy(osb[:], ops[:])
                        zc = epps.tile([128, QC], FP, tag="ep", name="ep")[0:N_HEAD, 0:cw]
                        matmul(zc, wsb["selZ"][:], osb[:], start=True, stop=True)
                        zi = sb.tile([N_HEAD, cw], FR, tag="o_zi")
                        with nc.allow_low_precision(reason="fp32r rounding of 1/Z"):
                            nc.vector.reciprocal(zi[:], zc)
                        zb = epps.tile([128, QC], FP, tag="ep", name="ep")[:, 0:cw]
                        matmul(zb, wsb["B4"][:], zi[:], start=True, stop=True)
                        osc = sb.tile([128, cw], FR, tag="o_osc")
                        nc.vector.tensor_mul(osc[:], osb[:], zb)
                        hd = epps.tile([128, QC], FP, tag="ep", name="ep")[0:D_MODEL, 0:cw]
                        matmul(hd, wsb[f"Wo_sp_{i}"][:], osc[:],
                               start=True, stop=False)
                        matmul(hd, wsb[f"bo_row_{i}"][:],
                               ones_row[:, 0:cw], start=False, stop=True)
                        dst = hT[:, out_lo + c * QC:out_lo + c * QC + cw]
                        nc.vector.tensor_add(dst, dst, hd)

            def mlp(i, lo, width):
                """h[:, lo:lo+width] += mlp(y_aug[:, lo:lo+width])."""
                with tc.tile_pool(name="mlps", bufs=2, space="PSUM") as mlps:
                    nch = (width + QC - 1) // QC
                    for c in range(nch):
                        o, wdt = lo + c * QC, min(QC, width - c * QC)
                        ps1 = mlps.tile([D_INNER, QC], FP, tag="mlp1", name="mlp1")[:, 0:wdt]
                        matmul(ps1, wsb[f"W1_aug_{i}"][:], y_aug[:, o:o + wdt],
                               start=True, stop=True)
                        nc.vector.tensor_scalar(r_mlp[:, o:o + wdt], ps1,
                                                0.0, None, op0=ALU.max)
                        ps2 = mlps.tile([D_MODEL, QC], FP, tag="mlp2", name="mlp2")[:, 0:wdt]
                        matmul(ps2, wsb[f"W2_{i}"][:], r_mlp[:, o:o + wdt],
                               start=True, stop=False)
                        matmul(ps2, wsb[f"b2_row_{i}"][:], ones_row[:, 0:wdt],
                               start=False, stop=True)
                        dst = hT[:, o:o + wdt]
                        nc.vector.tensor_add(dst, dst, ps2)

            # ---- layers ----
            for i in range(N_LAYERS):
                last = i == N_LAYERS - 1
                layernorm(hT, wsb[f"ln1_s_{i}"][:], wsb[f"ln1_b_{i}"][:], 0, T)
                project(f"Wk_aug_{i}", k_sp, 0, T)
                project(f"Wv_aug_{i}", v_sp, 0, T)
                build_vaug()
                if not last:
                    project(f"Wq_aug_{i}", q_sp, 0, T)
                    attention(i, q_sp, 0, T, 0)
                    layernorm(hT, wsb[f"ln2_s_{i}"][:], wsb[f"ln2_b_{i}"][:], 0, T)
                    mlp(i, 0, T)
                else:
                    project(f"Wq_aug_{i}", q3, T - 8, 8, dst_lo=0)
                    attention(i, q3, 0, 8, T - 8)
                    layernorm(hT, wsb[f"ln2_s_{i}"][:], wsb[f"ln2_b_{i}"][:], T - 1, 1)
                    mlp(i, T - 1, 1)

            # ---- final LN + head on last token ----
            layernorm(hT, wsb["lnf_s"][:], wsb["lnf_b"][:], T - 1, 1)
            with tc.tile_pool(name="headps", bufs=2, space="PSUM") as headps:
                ph1 = headps.tile([32, 512], FP, tag="head", name="head")[:, 0:1]
                matmul(ph1, wsb["Wh1_aug"][:], y_aug[:, T - 1:T],
                       start=True, stop=True)
                r1 = state.tile([33, 1], FP, tag="r1")
                nc.vector.memset(r1[32:33, :], 1.0)
                nc.scalar.activation(r1[0:32, :], ph1, AF.Relu)
                ph2 = headps.tile([1, 512], FP, tag="head", name="head")[:, 0:1]
                matmul(ph2, wsb["Wh2_aug"][:], r1[:], start=True, stop=True)
                osb = sb.tile([1, 1], FP, tag="out_sb")
                nc.vector.tensor_copy(osb[:], ph2)
                nc.sync.dma_start(out_dram[:], osb[:])

    nc.compile()
    return nc


def _get_program(weights):
    global _PROGRAM
    if _PROGRAM is None:
        _PROGRAM = _build_program({k: v.shape for k, v in weights.items()})
    return _PROGRAM


def kernel(**inputs):
    weights = _host_weights(inputs)
    nc = _get_program(weights)

    x = np.asarray(inputs["x"], dtype=np.float32)            # [8, 16, 2048]
    in_maps = []
    for b in range(N_CORES):
        xa = np.concatenate([x[b], np.ones((1, T), np.float32)], axis=0)
        in_maps.append({"x_aug": np.ascontiguousarray(xa), **weights})

    from concourse.bass_utils import run_bass_kernel_spmd
    res = run_bass_kernel_spmd(nc, in_maps, list(range(N_CORES)))
    out = np.stack([res.results[b]["out"].reshape(1) for b in range(N_CORES)], axis=0)
    return out.astype(np.float32)

